# revision 1
# baseline (speedup 1.0000x reference)
"""Trainium2 Bass kernel for nn_MultiHeadLatentAttention (MLA, causal, fp32).

Sharding (8 NeuronCores, no collectives): data-parallel. Core c handles batch
b = c // 4 and the balanced pair of 256-token query groups {g, 7-g}, g = c % 4.
Each core computes the kv-latent path and per-head K/V up-projections for the
full batch, the q-latent path for its own 512 query tokens, all 16 heads of
causal attention for its query groups, and the output projection - producing
complete rows of the output (host just scatters rows back).

To keep one SPMD program across cores despite per-core causal extents, the kv
token sequence is extended with duplicates of the two diagonal query groups
(fixed positions), invisible prefix k-chunks are zeroed via a per-core
additive exp-bias input (-1e9), and the triangular masks apply only at the
fixed diagonal slots.

Attention runs in the transposed layout S^T[k, q] end-to-end: scores, exp,
and the AV matmuls all keep k on partitions, so no PE transposes are needed;
softmax denominators come from ones-vector matmuls (partition-dim sums) and
the normalization is applied once to O^T columns.

All matmuls run in float32r (TF32-like; full PE rate at free-dim >= 256).
Host prep folds rms-norm weights and the 1/sqrt(64) score scale into adjacent
weights, and adds pair-swapped RoPE weight rows so the on-chip rotation is
pure same-base elementwise work.
"""
import sys
import numpy as np

sys.path.insert(0, '/opt/trn_rl_repo/concourse')
sys.path.insert(0, '/opt/trn_rl_repo')

import concourse.bass as bass  # noqa: E402,F401
import concourse.mybir as mybir  # noqa: E402
import concourse.tile as tile  # noqa: E402
from concourse import bacc  # noqa: E402
from contextlib import ExitStack, nullcontext  # noqa: E402

F32 = mybir.dt.float32
F32R = mybir.dt.float32r
P = 128
G = 256               # query-group width (2 subblocks of 128)
THETA = 10000.0
Mult = mybir.AluOpType.mult
Add = mybir.AluOpType.add


class Cfg:
    def __init__(self, B=2, T=2048, C=2048, H=16, QLR=1536, KVL=512,
                 ROPE=64, VD=128, n_cores=8, causal=True, bench_n=1,
                 bench_phases=(1, 2, 3, 5, 6)):
        self.B, self.T, self.C, self.H = B, T, C, H
        self.QLR, self.KVL, self.ROPE, self.VD = QLR, KVL, ROPE, VD
        self.n_cores, self.causal = n_cores, causal
        self.bench_n = bench_n
        self.bench_phases = tuple(bench_phases)
        self.NG = T // G                       # 256-groups per batch
        self.cpb = n_cores // B                # cores per batch
        assert self.NG == 2 * self.cpb
        self.TQ = 2 * G                        # q tokens per core
        self.TK = T + 2 * G                    # kv columns incl. diag dups
        self.CC = C // P
        self.RC = QLR // P
        self.KC = KVL // P

    @property
    def key(self):
        return (self.B, self.T, self.C, self.H, self.QLR, self.KVL,
                self.n_cores, self.causal, self.bench_n, self.bench_phases)


def core_groups(cfg, c):
    b = c // cfg.cpb
    g = c % cfg.cpb
    return b, [g, cfg.NG - 1 - g]


def rope_tables(positions, rope):
    """[2*rope, n] cos/sin tables for d-on-partitions layout; rows duplicated
    so both partition bases 0:rope and rope:2*rope hold the same table."""
    hr = rope // 2
    inv = 1.0 / THETA ** (np.arange(hr, dtype=np.float64) / hr)
    ang = positions[None, :].astype(np.float64) * inv[:, None]
    c, s = np.cos(ang), np.sin(ang)
    ct = np.empty((rope, len(positions)), np.float32)
    st = np.empty((rope, len(positions)), np.float32)
    ct[0::2] = c
    ct[1::2] = c
    st[0::2] = -s
    st[1::2] = s
    return np.concatenate([ct, ct], 0), np.concatenate([st, st], 0)


def pair_swap_rows(w):
    out = np.empty_like(w)
    out[0::2] = w[1::2]
    out[1::2] = w[0::2]
    return out


def ktiles(cfg, gi):
    """Attention k-tiles for group-slot gi: (slot, kv col offset, is_diag)."""
    if not cfg.causal:
        return [(kt, kt * G, False) for kt in range(cfg.NG)]
    nkp = (cfg.NG // 2 - 1) if gi == 0 else (cfg.NG - 1)
    tl = [(kt, kt * G, False) for kt in range(nkp)]
    tl.append((nkp, cfg.T + gi * G, True))
    return tl


def build_program(cfg):
    B, T, C, H = cfg.B, cfg.T, cfg.C, cfg.H
    QLR, KVL, ROPE, VD = cfg.QLR, cfg.KVL, cfg.ROPE, cfg.VD
    TQ, TK, CC, RC, KC, NG = cfg.TQ, cfg.TK, cfg.CC, cfg.RC, cfg.KC, cfg.NG
    NTT = TK // 512                  # 512-col tiles over kv sequence
    NTC = TK // P                    # 128-col chunks over kv sequence
    NP = H // 2
    NKV = KVL + 4 * ROPE             # wkva cols: [latent | rope|swap | swap|rope]
    KVG = NKV // P

    nc = bacc.Bacc(None, target_bir_lowering=False)

    def bloop(tc, phase=0):
        if cfg.bench_n > 1 and phase in cfg.bench_phases:
            return tc.For_i(0, cfg.bench_n, 1)
        return nullcontext()

    xq = nc.dram_tensor('xq', [C, TQ], F32, kind='ExternalInput')
    xkv = nc.dram_tensor('xkv', [C, TK], F32, kind='ExternalInput')
    wqa = nc.dram_tensor('wqa', [C, QLR], F32, kind='ExternalInput')
    wqb = nc.dram_tensor('wqb', [QLR, H * 192], F32, kind='ExternalInput')
    wkva = nc.dram_tensor('wkva', [C, NKV], F32, kind='ExternalInput')
    wkvbk = nc.dram_tensor('wkvbk', [KVL, NP * P], F32, kind='ExternalInput')
    wkvbv = nc.dram_tensor('wkvbv', [KVL, H * VD], F32, kind='ExternalInput')
    wo = nc.dram_tensor('wo', [H * VD, C], F32, kind='ExternalInput')
    ctq_d = nc.dram_tensor('ctq', [P, TQ], F32, kind='ExternalInput')
    stq_d = nc.dram_tensor('stq', [P, TQ], F32, kind='ExternalInput')
    ctk_d = nc.dram_tensor('ctk', [P, TK], F32, kind='ExternalInput')
    stk_d = nc.dram_tensor('stk', [P, TK], F32, kind='ExternalInput')
    masksT_d = nc.dram_tensor('masksT', [P, 2 * G], F32, kind='ExternalInput')
    visb_d = nc.dram_tensor('visb', [P, 2 * NG], F32, kind='ExternalInput')
    ones_d = nc.dram_tensor('ones_in', [P, P], F32, kind='ExternalInput')
    y_out = nc.dram_tensor('y', [TQ, C], F32, kind='ExternalOutput')

    with tile.TileContext(nc) as tc, ExitStack() as top:
        const_p = top.enter_context(tc.tile_pool(name='const', bufs=1))
        dram_p = top.enter_context(tc.tile_pool(name='dram', bufs=1, space='DRAM'))
        # manually-scoped pools, strictly LIFO: big_p(ph2..ph5) > qc_p(ph1..ph3)
        big_cm = tc.tile_pool(name='big_p', bufs=1)
        qc_cm = tc.tile_pool(name='qc_p', bufs=1)

        masksT = const_p.tile([P, 2 * G], F32)
        nc.sync.dma_start(masksT[:], masksT_d[:])
        visb = const_p.tile([P, 2 * NG], F32)
        nc.sync.dma_start(visb[:], visb_d[:])
        ctq = const_p.tile([P, TQ], F32)
        stq = const_p.tile([P, TQ], F32)
        nc.sync.dma_start(ctq[:], ctq_d[:])
        nc.sync.dma_start(stq[:], stq_d[:])
        ones_k = const_p.tile([P, 1], F32R)          # [128,1] for partition sums
        nc.sync.dma_start(ones_k[:], ones_d[:, 0:1].bitcast(F32R))
        ones_b = const_p.tile([1, P], F32R)          # [1,128] for broadcasts
        nc.sync.dma_start(ones_b[:], ones_d[0:1, :].bitcast(F32R))

        big_p = big_cm.__enter__()
        kv_c = [big_p.tile([P, TK], F32R, tag=f'kv_c{i}', name=f'kv_c{i}')
                for i in range(KC)]
        k_rot = big_p.tile([P, TK], F32R, tag='k_rot')
        qhat = big_p.tile([P, H, TQ], F32R, tag='qhat')
        ot_dram = dram_p.tile([H, P, TQ], F32)

        # ============ Phase 2: kv latent + norm + shared rope key =========
        with tc.tile_pool(name='ph2', bufs=1) as ph2, \
             tc.tile_pool(name='ph2s', bufs=2) as ph2s, \
             tc.tile_pool(name='ps2l', bufs=3, space='PSUM') as ps2l, \
             tc.tile_pool(name='ps2r', bufs=3, space='PSUM') as ps2r, \
             tc.tile_pool(name='ps2m', bufs=1, space='PSUM') as ps2m, \
             bloop(tc, 2):
            wkva_t = ph2.tile([P, CC, NKV], F32R, tag='wkva')
            nc.sync.dma_start(
                wkva_t[:], wkva[:].rearrange('(cc p) r -> p cc r', p=P).bitcast(F32R))
            ctk = ph2.tile([P, TK], F32, tag='ctk')
            stk = ph2.tile([P, TK], F32, tag='stk')
            nc.sync.dma_start(ctk[:], ctk_d[:])
            nc.sync.dma_start(stk[:], stk_d[:])
            for tt in range(NTT):
                tsl = slice(tt * 512, (tt + 1) * 512)
                xkv_t = ph2.tile([P, CC, 512], F32R, tag='xkv')
                nc.sync.dma_start(
                    xkv_t[:],
                    xkv[:, tsl].rearrange('(cc p) t -> p cc t', p=P).bitcast(F32R))
                msq = ps2m.tile([1, 512], F32, tag='kmsq')
                for g in range(KC):
                    acc = ps2l.tile([P, 512], F32, tag='kvlat')
                    for cc in range(CC):
                        nc.tensor.matmul(acc[:], wkva_t[:, cc, g * P:(g + 1) * P],
                                         xkv_t[:, cc, :], start=(cc == 0),
                                         stop=(cc == CC - 1))
                    nc.vector.tensor_copy(kv_c[g][:, tsl], acc[:])
                    sq = ph2s.tile([P, 512], F32R, tag='ksq')
                    nc.vector.tensor_tensor(sq[:], kv_c[g][:, tsl],
                                            kv_c[g][:, tsl], Mult)
                    nc.tensor.matmul(msq[:], ones_k[:], sq[:],
                                     start=(g == 0), stop=(g == KC - 1))
                ropes = []
                for g in range(KC, KVG):
                    acc = ps2r.tile([P, 512], F32, tag='kvrope')
                    for cc in range(CC):
                        nc.tensor.matmul(acc[:], wkva_t[:, cc, g * P:(g + 1) * P],
                                         xkv_t[:, cc, :], start=(cc == 0),
                                         stop=(cc == CC - 1))
                    ropes.append(acc)
                g4, g5 = ropes           # g4 = [rope|swap], g5 = [swap|rope]
                tmp = ph2s.tile([P, 512], F32R, tag='krt')
                nc.vector.tensor_tensor(k_rot[0:64, tsl], g4[0:64, :],
                                        ctk[0:64, tsl], Mult)
                nc.vector.tensor_tensor(tmp[0:64, :], g5[0:64, :],
                                        stk[0:64, tsl], Mult)
                nc.vector.tensor_tensor(k_rot[0:64, tsl], k_rot[0:64, tsl],
                                        tmp[0:64, :], Add)
                nc.vector.tensor_tensor(k_rot[64:P, tsl], g5[64:P, :],
                                        ctk[64:P, tsl], Mult)
                nc.vector.tensor_tensor(tmp[64:P, :], g4[64:P, :],
                                        stk[64:P, tsl], Mult)
                nc.vector.tensor_tensor(k_rot[64:P, tsl], k_rot[64:P, tsl],
                                        tmp[64:P, :], Add)
                rs_sb = ph2s.tile([1, 512], F32, tag='krs')
                nc.vector.tensor_scalar(rs_sb[:], msq[:], 1.0 / KVL, 1e-6, Mult, Add)
                nc.scalar.activation(rs_sb[:], rs_sb[:],
                                     mybir.ActivationFunctionType.Sqrt)
                nc.vector.reciprocal(rs_sb[:], rs_sb[:])
                rs_r = ph2s.tile([1, 512], F32R, tag='krsr')
                nc.vector.tensor_copy(rs_r[:], rs_sb[:])
                rs_bc = ps2m.tile([P, 512], F32, tag='krsbc')
                nc.tensor.matmul(rs_bc[:], ones_b[:], rs_r[:], start=True, stop=True)
                for g in range(KC):
                    nc.vector.tensor_tensor(kv_c[g][:, tsl], kv_c[g][:, tsl],
                                            rs_bc[:], Mult)

        # ================= Phase 1: q latent + rms norm ===================
        qc_p = qc_cm.__enter__()
        q_c = [qc_p.tile([P, TQ], F32R, tag=f'q_c{i}', name=f'q_c{i}')
               for i in range(RC)]
        with tc.tile_pool(name='ph1', bufs=1) as ph1, \
             tc.tile_pool(name='ph1s', bufs=3) as ph1s, \
             tc.tile_pool(name='ps1', bufs=3, space='PSUM') as ps1, \
             tc.tile_pool(name='ps1m', bufs=1, space='PSUM') as ps1m, \
             bloop(tc, 1):
            xq_t = ph1.tile([P, CC, TQ], F32R, tag='xq')
            nc.sync.dma_start(
                xq_t[:], xq[:].rearrange('(cc p) t -> p cc t', p=P).bitcast(F32R))
            msq = ps1m.tile([1, TQ], F32, tag='msq')
            for rc in range(RC):
                wqa_t = ph1s.tile([P, CC, P], F32R, tag='wqa')
                nc.sync.dma_start(
                    wqa_t[:], wqa[:, rc * P:(rc + 1) * P]
                    .rearrange('(cc p) r -> p cc r', p=P).bitcast(F32R))
                acc = ps1.tile([P, TQ], F32, tag='qacc')
                for cc in range(CC):
                    nc.tensor.matmul(acc[:], wqa_t[:, cc, :],
                                     xq_t[:, cc, :], start=(cc == 0),
                                     stop=(cc == CC - 1))
                nc.vector.tensor_copy(q_c[rc][:], acc[:])
                sq = ph1s.tile([P, TQ], F32R, tag='sq')
                nc.vector.tensor_tensor(sq[:], q_c[rc][:], q_c[rc][:], Mult)
                nc.tensor.matmul(msq[:], ones_k[:], sq[:],
                                 start=(rc == 0), stop=(rc == RC - 1))
            rs_sb = ph1s.tile([1, TQ], F32, tag='rs')
            nc.vector.tensor_scalar(rs_sb[:], msq[:], 1.0 / QLR, 1e-6, Mult, Add)
            nc.scalar.activation(rs_sb[:], rs_sb[:],
                                 mybir.ActivationFunctionType.Sqrt)
            nc.vector.reciprocal(rs_sb[:], rs_sb[:])
            rs_r = ph1s.tile([1, TQ], F32R, tag='rsr')
            nc.vector.tensor_copy(rs_r[:], rs_sb[:])
            rs_bc = ps1m.tile([P, TQ], F32, tag='rsbc')
            nc.tensor.matmul(rs_bc[:], ones_b[:], rs_r[:], start=True, stop=True)
            for rc in range(RC):
                nc.vector.tensor_tensor(q_c[rc][:], q_c[rc][:], rs_bc[:], Mult)

        # ================= Phase 3: per-head q-hat (rotated) ==============
        # wqb cols per head: even h: [rope64 | nope64 | swap64]
        #                    odd  h: [nope64 | rope64 | swap64]
        with tc.tile_pool(name='ph3s', bufs=2) as ph3s, \
             tc.tile_pool(name='ps3', bufs=2, space='PSUM') as ps3, \
             bloop(tc, 3):
            for h in range(H):
                wqb_t = ph3s.tile([P, RC, 192], F32R, tag='wqb')
                nc.sync.dma_start(
                    wqb_t[:], wqb[:, h * 192:(h + 1) * 192]
                    .rearrange('(rc p) d -> p rc d', p=P).bitcast(F32R))
                ps_m = ps3.tile([P, TQ], F32, tag='qb_m')
                ps_s = ps3.tile([P, TQ], F32, tag='qb_s')
                rb = 0 if h % 2 == 0 else 64
                for rc in range(RC):
                    nc.tensor.matmul(ps_m[:], wqb_t[:, rc, 0:P], q_c[rc][:],
                                     start=(rc == 0), stop=(rc == RC - 1))
                    nc.tensor.matmul(ps_s[0:64, :], wqb_t[:, rc, P:192],
                                     q_c[rc][:], start=(rc == 0),
                                     stop=(rc == RC - 1))
                qh = qhat[:, h, :]
                nb = 64 - rb
                nc.vector.tensor_copy(qh[nb:nb + 64, :], ps_m[nb:nb + 64, :])
                rr = slice(rb, rb + 64)
                tmp = ph3s.tile([P, TQ], F32R, tag='qrt')
                nc.vector.tensor_tensor(qh[rr, :], ps_m[rr, :], ctq[rr, :], Mult)
                nc.vector.tensor_tensor(tmp[rr, :], ps_s[0:64, :], stq[rr, :], Mult)
                nc.vector.tensor_tensor(qh[rr, :], qh[rr, :], tmp[rr, :], Add)
        qc_cm.__exit__(None, None, None)

        # ========== Phase 4/5: per head-pair K/V + attention (S^T) ========
        with tc.tile_pool(name='ph5', bufs=1) as ph5, \
             tc.tile_pool(name='ph5b', bufs=2) as ph5b, \
             tc.tile_pool(name='ps5p', bufs=2, space='PSUM') as ps5p, \
             tc.tile_pool(name='ps5s', bufs=2, space='PSUM') as ps5s, \
             tc.tile_pool(name='ps5r', bufs=2, space='PSUM') as ps5r, \
             tc.tile_pool(name='ps5o', bufs=2, space='PSUM') as ps5o, \
             bloop(tc, 5):
            for pair in range(NP):
                h0, h1 = 2 * pair, 2 * pair + 1
                v_pair = ph5.tile([P, NTC, 2 * VD], F32R, tag='v_pair')
                wv_t = ph5b.tile([P, KC, 2 * VD], F32R, tag='wv')
                nc.sync.dma_start(
                    wv_t[:], wkvbv[:, h0 * VD:(h0 + 2) * VD]
                    .rearrange('(kc p) d -> p kc d', p=P).bitcast(F32R))
                for tci in range(NTC):
                    ps_v = ps5p.tile([P, 512], F32, tag='prep')
                    for kc in range(KC):
                        nc.tensor.matmul(ps_v[:, 0:2 * VD],
                                         kv_c[kc][:, tci * P:(tci + 1) * P],
                                         wv_t[:, kc, :], start=(kc == 0),
                                         stop=(kc == KC - 1))
                    nc.vector.tensor_copy(v_pair[:, tci, :], ps_v[:, 0:2 * VD])
                # wkvbk pair cols: [content(h1) 64 | content(h0) 64]
                kT = [ph5.tile([P, TK], F32R, tag='kT0', name='kT0'),
                      ph5.tile([P, TK], F32R, tag='kT1', name='kT1')]
                wk_t = ph5b.tile([P, KC, P], F32R, tag='wk')
                nc.sync.dma_start(
                    wk_t[:], wkvbk[:, pair * P:(pair + 1) * P]
                    .rearrange('(kc p) d -> p kc d', p=P).bitcast(F32R))
                for tt in range(NTT):
                    tsl = slice(tt * 512, (tt + 1) * 512)
                    ps_k = ps5p.tile([P, 512], F32, tag='prep')
                    for kc in range(KC):
                        nc.tensor.matmul(ps_k[:], wk_t[:, kc, :], kv_c[kc][:, tsl],
                                         start=(kc == 0), stop=(kc == KC - 1))
                    # even head h0: [rope 0:64 | content 64:128]
                    nc.vector.tensor_copy(kT[0][64:P, tsl], ps_k[64:P, :])
                    nc.vector.tensor_copy(kT[0][0:64, tsl], k_rot[0:64, tsl])
                    # odd head h1: [content 0:64 | rope 64:128]
                    nc.vector.tensor_copy(kT[1][0:64, tsl], ps_k[0:64, :])
                    nc.vector.tensor_copy(kT[1][64:P, tsl], k_rot[64:P, tsl])
                for hl, h in enumerate((h0, h1)):
                    for gi in range(2):
                        tl = ktiles(cfg, gi)
                        qsl = slice(gi * G, (gi + 1) * G)
                        # exp'd S^T chunks [k 128, q G] per 128-col k chunk
                        pt_un = ph5.tile([P, 2 * NG, G], F32R, tag='pt_un')
                        rs_ps = ps5r.tile([1, G], F32, tag='rsb')
                        ot_ps = ps5o.tile([P, G], F32, tag='ot')
                        chunks = [(slot, koff, j, isdiag)
                                  for (slot, koff, isdiag) in tl for j in range(2)]
                        nch = len(chunks)
                        for ci, (slot, koff, j, isdiag) in enumerate(chunks):
                            s_ps = ps5s.tile([P, G], F32, tag='s')
                            nc.tensor.matmul(
                                s_ps[:], kT[hl][:, koff + j * P:koff + (j + 1) * P],
                                qhat[:, h, qsl], start=True, stop=True)
                            if isdiag:
                                nc.vector.tensor_tensor(
                                    s_ps[:], s_ps[:], masksT[:, j * G:(j + 1) * G],
                                    Add)
                            pu = pt_un[:, 2 * slot + j, :]
                            nc.scalar.activation(
                                pu, s_ps[:], mybir.ActivationFunctionType.Exp,
                                bias=visb[:, gi * NG + slot:gi * NG + slot + 1])
                            nc.tensor.matmul(rs_ps[:], ones_k[:], pu,
                                             start=(ci == 0), stop=(ci == nch - 1))
                            nc.tensor.matmul(
                                ot_ps[:], v_pair[:, koff // P + j,
                                                 hl * VD:(hl + 1) * VD],
                                pu, start=(ci == 0), stop=(ci == nch - 1))
                        rec = ph5b.tile([1, G], F32, tag='rec')
                        nc.vector.reciprocal(rec[:], rs_ps[:])
                        rec_r = ph5b.tile([1, G], F32R, tag='recr')
                        nc.vector.tensor_copy(rec_r[:], rec[:])
                        rb_ps = ps5r.tile([P, G], F32, tag='rsb')
                        nc.tensor.matmul(rb_ps[:], ones_b[:], rec_r[:],
                                         start=True, stop=True)
                        rb_sb = ph5b.tile([P, G], F32, tag='rbsb')
                        nc.vector.tensor_copy(rb_sb[:], rb_ps[:])
                        ot_sb = ph5b.tile([P, G], F32, tag='otsb')
                        nc.vector.tensor_tensor(ot_sb[:], ot_ps[:], rb_sb[:], Mult)
                        nc.sync.dma_start(ot_dram[h, :, qsl], ot_sb[:])
        big_cm.__exit__(None, None, None)

        # ================= Phase 6: output projection =====================
        NCT = C // 512
        NTS = TQ // P
        with tc.tile_pool(name='ph6', bufs=1) as ph6, \
             tc.tile_pool(name='ph6w', bufs=2) as ph6w, \
             tc.tile_pool(name='ph6o', bufs=3) as ph6o, \
             tc.tile_pool(name='ps6', bufs=3, space='PSUM') as ps6, \
             bloop(tc, 6):
            ot_all = ph6.tile([P, H, TQ], F32R, tag='ot_all')
            nc.sync.dma_start(ot_all[:],
                              ot_dram[:].rearrange('h p t -> p h t').bitcast(F32R))
            for ct in range(NCT):
                wo_t = ph6w.tile([P, H, 512], F32R, tag='wo')
                nc.sync.dma_start(
                    wo_t[:], wo[:, ct * 512:(ct + 1) * 512]
                    .rearrange('(h p) c -> p h c', p=P).bitcast(F32R))
                for ts in range(NTS):
                    ps_y = ps6.tile([P, 512], F32, tag='psy')
                    for h in range(H):
                        nc.tensor.matmul(ps_y[:], ot_all[:, h, ts * P:(ts + 1) * P],
                                         wo_t[:, h, :], start=(h == 0),
                                         stop=(h == H - 1))
                    y_sb = ph6o.tile([P, 512], F32, tag='ysb')
                    nc.vector.tensor_copy(y_sb[:], ps_y[:])
                    nc.sync.dma_start(
                        y_out[ts * P:(ts + 1) * P, ct * 512:(ct + 1) * 512], y_sb[:])

    nc.compile()
    return nc


def host_prepare(cfg, x, w_qa, qa_norm_w, w_qb, w_kva, kva_norm_w, w_kvb, w_o):
    """Build per-core input maps. Returns (in_maps, scatter_info)."""
    B, T, C, H = cfg.B, cfg.T, cfg.C, cfg.H
    QLR, KVL, ROPE, VD, NG = cfg.QLR, cfg.KVL, cfg.ROPE, cfg.VD, cfg.NG
    f32 = np.float32
    scale = np.float32(ROPE ** -0.5)

    wqa_h = np.ascontiguousarray(w_qa.T).astype(f32)
    wqb_s = (w_qb.astype(np.float64) * qa_norm_w[None, :]).astype(f32) * scale
    wqb_h = np.empty((QLR, H * 192), f32)
    for h in range(H):
        blk = wqb_s[h * 2 * ROPE:(h + 1) * 2 * ROPE]
        nope, rope = blk[:ROPE], blk[ROPE:]
        sw = pair_swap_rows(rope)
        main = np.concatenate([rope, nope], 0) if h % 2 == 0 \
            else np.concatenate([nope, rope], 0)
        wqb_h[:, h * 192:h * 192 + P] = main.T
        wqb_h[:, h * 192 + P:(h + 1) * 192] = sw.T
    kva_lat, kva_rope = w_kva[:KVL], w_kva[KVL:]
    kva_sw = pair_swap_rows(kva_rope)
    wkva_h = np.ascontiguousarray(
        np.concatenate([kva_lat, kva_rope, kva_sw, kva_sw, kva_rope], 0).T
    ).astype(f32)
    wkvb_s = (w_kvb.astype(np.float64) * kva_norm_w[None, :]).astype(f32)
    wkvbk_h = np.empty((KVL, (H // 2) * P), f32)
    wkvbv_h = np.empty((KVL, H * VD), f32)
    for h in range(H):
        blk = wkvb_s[h * (ROPE + VD):(h + 1) * (ROPE + VD)]
        kcont, v = blk[:ROPE], blk[ROPE:]
        wkvbv_h[:, h * VD:(h + 1) * VD] = v.T
        pair = h // 2
        if h % 2 == 1:
            wkvbk_h[:, pair * P:pair * P + ROPE] = kcont.T
        else:
            wkvbk_h[:, pair * P + ROPE:(pair + 1) * P] = kcont.T
    wo_h = np.ascontiguousarray(w_o.T).astype(f32)

    ones = np.ones((P, P), f32)
    NEG = np.float32(-1e9)
    # transposed diagonal masks [k within chunk, q within group], chunk j=0/1:
    # visible iff q >= j*128 + k
    mT = np.zeros((P, 2 * G), f32)
    for k in range(P):
        mT[k, 0:G] = np.where(np.arange(G) >= k, 0.0, NEG)
        mT[k, G:2 * G] = np.where(np.arange(G) >= P + k, 0.0, NEG)

    xf = x.reshape(B, T, C)
    in_maps, scat = [], []
    for c in range(cfg.n_cores):
        b, grps = core_groups(cfg, c)
        toks = np.concatenate([np.arange(g * G, (g + 1) * G) for g in grps])
        ktoks = np.concatenate([np.arange(T)] +
                               [np.arange(g * G, (g + 1) * G) for g in grps])
        xq_h = np.ascontiguousarray(xf[b][toks].T).astype(f32)
        xkv_h = np.ascontiguousarray(xf[b][ktoks].T).astype(f32)
        ctq_h, stq_h = rope_tables(toks, ROPE)
        ctk_h, stk_h = rope_tables(ktoks, ROPE)
        # exp bias per (gi, slot): 0 for visible prefix groups and the diag,
        # -1e9 for invisible prefix slots
        vb = np.zeros((P, 2 * NG), f32)
        for gi, g in enumerate(grps):
            for (slot, koff, isdiag) in ktiles(cfg, gi):
                if not isdiag and (koff // G) >= g:
                    vb[:, gi * NG + slot] = NEG
        in_maps.append({
            'xq': xq_h, 'xkv': xkv_h, 'wqa': wqa_h, 'wqb': wqb_h,
            'wkva': wkva_h, 'wkvbk': wkvbk_h, 'wkvbv': wkvbv_h, 'wo': wo_h,
            'ctq': ctq_h, 'stq': stq_h, 'ctk': ctk_h, 'stk': stk_h,
            'masksT': mT, 'visb': vb, 'ones_in': ones,
        })
        scat.append((b, toks))
    return in_maps, scat


_NC_CACHE = {}
_RUNNER_CACHE = {}


def get_program(cfg):
    if cfg.key not in _NC_CACHE:
        _NC_CACHE[cfg.key] = build_program(cfg)
    return _NC_CACHE[cfg.key]


def make_runner(nc, n_cores):
    """Build a reusable jitted SPMD executor for a compiled Bass program
    (mirrors concourse.bass2jax.run_bass_via_pjrt, but caches the jit)."""
    import jax
    from jax.sharding import Mesh, PartitionSpec
    from jax.experimental.shard_map import shard_map
    from concourse.bass2jax import (_bass_exec_p, install_neuronx_cc_hook,
                                    partition_id_tensor)
    install_neuronx_cc_hook()
    partition_name = nc.partition_id_tensor.name if nc.partition_id_tensor else None
    in_names, out_names, out_avals, zero_outs = [], [], [], []
    for alloc in nc.m.functions[0].allocations:
        if not isinstance(alloc, mybir.MemoryLocationSet):
            continue
        name = alloc.memorylocations[0].name
        if alloc.kind == 'ExternalInput':
            if name != partition_name:
                in_names.append(name)
        elif alloc.kind == 'ExternalOutput':
            np_dt = mybir.dt.np(alloc.dtype)
            out_avals.append(jax.core.ShapedArray(tuple(alloc.tensor_shape), np_dt))
            out_names.append(name)
            zero_outs.append(np.zeros(tuple(alloc.tensor_shape), np_dt))
    n_params = len(in_names)
    n_outs = len(out_names)
    in_names.extend(out_names)
    if partition_name is not None:
        in_names.append(partition_name)

    def _body(*args):
        operands = list(args)
        if partition_name is not None:
            operands.append(partition_id_tensor())
        outs = _bass_exec_p.bind(
            *operands, out_avals=tuple(out_avals), in_names=tuple(in_names),
            out_names=tuple(out_names), lowering_input_output_aliases=(),
            sim_require_finite=True, sim_require_nnan=True, nc=nc)
        return tuple(outs)

    devices = jax.devices()[:n_cores]
    mesh = Mesh(np.asarray(devices), ('core',))
    in_specs = (PartitionSpec('core'),) * (n_params + n_outs)
    out_specs = (PartitionSpec('core'),) * n_outs
    fn = jax.jit(shard_map(_body, mesh=mesh, in_specs=in_specs,
                           out_specs=out_specs, check_rep=False),
                 keep_unused=True)

    def prepare(in_maps):
        import jax
        concat_in = [np.concatenate([in_maps[c][nm] for c in range(n_cores)],
                                    axis=0) for nm in in_names[:n_params]]
        concat_zeros = [np.concatenate([z] * n_cores, axis=0) for z in zero_outs]
        return [jax.device_put(a) for a in concat_in + concat_zeros]

    def execute(args):
        import jax
        outs = fn(*args)
        jax.block_until_ready(outs)
        res = []
        for c in range(n_cores):
            d = {}
            for i, nm in enumerate(out_names):
                full = np.asarray(outs[i])
                per = full.shape[0] // n_cores
                d[nm] = full[c * per:(c + 1) * per]
            res.append(d)
        return res

    def run(in_maps):
        return execute(prepare(in_maps))
    run.prepare = prepare
    run.execute = execute
    return run


def get_runner(cfg):
    if cfg.key not in _RUNNER_CACHE:
        _RUNNER_CACHE[cfg.key] = make_runner(get_program(cfg), cfg.n_cores)
    return _RUNNER_CACHE[cfg.key]


def run_cfg(cfg, x, w_qa, qa_norm_w, w_qb, w_kva, kva_norm_w, w_kvb, w_o):
    run = get_runner(cfg)
    in_maps, scat = host_prepare(cfg, x, w_qa, qa_norm_w, w_qb, w_kva,
                                 kva_norm_w, w_kvb, w_o)
    results = run(in_maps)
    y = np.zeros((cfg.B, cfg.T, cfg.C), np.float32)
    for c in range(cfg.n_cores):
        b, toks = scat[c]
        y[b][toks] = results[c]['y']
    return y


def kernel(x, mask, w_qa, qa_norm_w, w_qb, w_kva, kva_norm_w, w_kvb, w_o):
    x = np.asarray(x, np.float32)
    B, T, C = x.shape
    cfg = Cfg(B=B, T=T, C=C, H=16, QLR=w_qa.shape[0], KVL=512, ROPE=64,
              VD=128, n_cores=8, causal=True)
    mk = np.asarray(mask)[0, 0]
    if not np.array_equal(mk, np.tril(np.ones((T, T), np.int32))):
        if np.all(mk == 1):
            cfg.causal = False
        else:
            return _numpy_fallback(x, mk, w_qa, qa_norm_w, w_qb, w_kva,
                                   kva_norm_w, w_kvb, w_o)
    return run_cfg(cfg, x, w_qa, qa_norm_w, w_qb, w_kva, kva_norm_w,
                   w_kvb, w_o)


def _numpy_fallback(x, mk, w_qa, qa_norm_w, w_qb, w_kva, kva_norm_w, w_kvb, w_o):
    B, T, C = x.shape
    H, ROPE, KVL, VD = 16, 64, 512, 128

    def rms(v, w):
        return v / np.sqrt((v * v).mean(-1, keepdims=True) + 1e-6) * w

    q_c = rms(x @ w_qa.T, qa_norm_w)
    q = (q_c @ w_qb.T).reshape(B, T, H, 2 * ROPE)
    kv_a = x @ w_kva.T
    kv_c = rms(kv_a[..., :KVL], kva_norm_w)
    k_rope_raw = kv_a[..., KVL:]
    kv = (kv_c @ w_kvb.T).reshape(B, T, H, ROPE + VD)
    k_c, v = kv[..., :ROPE], kv[..., ROPE:]
    hr = ROPE // 2
    inv = 1.0 / THETA ** (np.arange(hr) / hr)
    ang = np.arange(T)[:, None] * inv
    cos, sin = np.cos(ang), np.sin(ang)

    def rot(t):
        xr, xi = t[..., 0::2], t[..., 1::2]
        c = cos[None, :, None, :]
        s = sin[None, :, None, :]
        return np.stack([xr * c - xi * s, xr * s + xi * c], -1).reshape(t.shape)

    q_r = rot(q[..., ROPE:])
    k_r = rot(np.broadcast_to(k_rope_raw[:, :, None, :], (B, T, H, ROPE)).copy())
    qq = np.concatenate([q[..., :ROPE], q_r], -1)
    kk = np.concatenate([k_c, k_r], -1)
    att = np.einsum('bqhd,bkhd->bhqk', qq, kk) * ROPE ** -0.5
    att = np.where(mk[None, None] == 0, -1e30, att)
    att = att - att.max(-1, keepdims=True)
    att = np.exp(att)
    att = att / att.sum(-1, keepdims=True)
    out = np.einsum('bhqk,bkhd->bqhd', att, v).reshape(B, T, H * VD)
    return (out @ w_o.T).astype(np.float32)

